# revision 10
# baseline (speedup 1.0000x reference)
"""Trainium2 Bass kernel for depthwise-spatial-conv:
out[b, i*D+d, 0, t] = sum_c maxnorm(w)[i*D+d, c] * x[b, i, c, t]

Sharding: data-parallel over batch (B=32 -> 4 per core across 8 cores),
weight replicated on every core.

Per core, each (b, i) is a tiny (8 x 128) @ (128 x 2048) fp32 matmul.
Structure: i-blocks are processed in groups of 4 via block-diagonal
(C x 32) weights, so each 4-matmul PSUM accumulation group yields a dense
(32, 512) tile at a 32-aligned partition base (engine partition bases must
be 32-aligned). Groups touch exactly one x DMA-tile, keeping accumulation
chains short and letting DMA/PE/DVE pipeline freely.

Matmul operands are bitcast to float32r: same fp32 bits, but the PE
processes the moving operand at 1 cycle/row (vs 4 for strict fp32) when
the free dim >= 256, dropping PE busy-time under the DMA floor.
"""
import numpy as np

import concourse.bacc as bacc
import concourse.mybir as mybir
import concourse.tile as tile
from concourse.bass_utils import run_bass_kernel_spmd
from concourse.masks import make_identity

F32 = mybir.dt.float32
F32R = mybir.dt.float32r

B, I, C, T, D = 32, 16, 128, 2048, 8
OUT_CH = I * D  # 128
N_CORES = 8
BPC = B // N_CORES  # batches per core
IG = 4            # i-blocks per DMA tile (4 MiB per load) and per psum group
N_IG = I // IG    # 4
JT = 512          # matmul moving free-dim chunk (psum bank limit for f32 out)
N_J = T // JT     # 4

_CACHE = {}


def _preprocess_weights(nc, wp, pp, w):
    """DMA w, transpose to wT[c, oc] (unscaled), and compute the torch
    renorm(p=2, dim=0, maxnorm=1) scale as a per-out-channel (128,1)
    vector. The scale is applied during the PSUM-drain copies, so the
    sqrt/ACT-table chain stays off the first-matmul critical path."""
    w_sb = wp.tile([OUT_CH, C], F32)
    # ACT ring: keep the SP ring free so the first x load issues immediately
    nc.scalar.dma_start(out=w_sb[:, :], in_=w[:, 0, :, 0])
    sq = wp.tile([OUT_CH, C], F32)
    nc.vector.tensor_mul(sq[:, :], w_sb[:, :], w_sb[:, :])
    norm2 = wp.tile([OUT_CH, 1], F32)
    nc.vector.reduce_sum(out=norm2[:, :], in_=sq[:, :],
                         axis=mybir.AxisListType.X)
    norm = wp.tile([OUT_CH, 1], F32)
    nc.scalar.activation(out=norm[:, :], in_=norm2[:, :],
                         func=mybir.ActivationFunctionType.Sqrt,
                         bias=0.0, scale=1.0)
    nc.vector.tensor_scalar_max(norm[:, :], norm[:, :], 1e-12)
    inv = wp.tile([OUT_CH, 1], F32)
    nc.vector.reciprocal(inv[:, :], norm[:, :])
    nc.vector.tensor_scalar_min(inv[:, :], inv[:, :], 1.0)
    ident = wp.tile([128, 128], F32)
    make_identity(nc, ident[:, :])
    pt = pp.tile([128, 128], F32, tag="ps", bufs=8)
    nc.tensor.transpose(pt[:, :], w_sb[:, :], ident[:, :])
    return pt, inv


def _blockdiag4(nc, wp, wT, dtype, name):
    """t[:, i, :] is (C, 32): cols [8*(i%4), 8*(i%4)+8) = wT[:, 8i:8i+8),
    zero elsewhere. A 4-matmul accumulation over i in one group fills a
    dense (32, JT) psum tile."""
    t = wp.tile([C, I, 32], dtype, name=name)
    nc.vector.memset(t[:, :, :], 0.0)
    for i in range(I):
        m = i % IG
        nc.vector.tensor_copy(t[:, i, m * D:(m + 1) * D],
                              wT[:, i * D:(i + 1) * D])
    return t


def _blockdiag_full(nc, wp, wT, name):
    """t[:, i, :] is (C, 128): cols [8i, 8i+8) = wT[:, 8i:8i+8), zero
    elsewhere. A 16-matmul full-width accumulation over i fills a dense
    (128, JT) psum tile — required for float32r, whose fast single-pass
    'HIGH' mode only exists at col_grp 0xf (all four 32-col groups).
    Built in fp32 (DVE can't write float32r), then bounced through a
    1 MiB SBUF->SBUF DMA into a float32r-tagged tile: walrus accepts a
    DMACopy with fp32r output as 'rounded', which the fp32r matmult
    input check requires."""
    t0 = wp.tile([C, I, 128], F32, name=name + "0")
    nc.vector.memset(t0[:, :, :], 0.0)
    for i in range(I):
        nc.vector.tensor_copy(t0[:, i, i * D:(i + 1) * D],
                              wT[:, i * D:(i + 1) * D])
    t = wp.tile([C, I, 128], F32R, name=name)
    nc.sync.dma_start(out=t[:, :, :], in_=t0[:, :, :].bitcast(F32R))
    return t


def _body(nc, tc, x, w, o, mm_dt=F32R):
    with tc.tile_pool(name="wp", bufs=1) as wp, \
         tc.tile_pool(name="xp", bufs=5) as xp, \
         tc.tile_pool(name="op", bufs=3) as op, \
         tc.tile_pool(name="pp", bufs=1, space="PSUM") as pp:
        wT, scale = _preprocess_weights(nc, wp, pp, w)
        if mm_dt == F32R:
            wbd = _blockdiag_full(nc, wp, wT, "wbd")
        else:
            wbd4 = _blockdiag4(nc, wp, wT, mm_dt, "wbd4")

        # PE warm-up: HAM throttles a cold PE to 1.2 GHz until ~3.4us of
        # sustained matmul activity. Burn that window during the initial
        # DMA fill with dummy matmuls (identity inputs, result unused) so
        # the real stream starts at full clock.
        wdum = wp.tile([128, 128], F32, name="wdum")
        nc.vector.memset(wdum[:, :], 0.5)
        psd = pp.tile([32, 128], F32, name="psd", tag="ps", bufs=8)
        for _ in range(12):
            nc.tensor.matmul(psd[:, :], wdum[:, :32], wdum[:, :],
                             start=True, stop=True)

        for b in range(BPC):
            out_sb = op.tile([OUT_CH, T], F32, name="out_sb", tag="ob")
            if mm_dt == F32R:
                # g-outer: each 4 MiB x-tile is fully consumed by its own
                # 16 matmuls (4 j-banks x 4 m) right after it lands, so the
                # tile frees early and the DMA stream never stalls on the
                # batch tail. The 4 psum banks accumulate across g
                # (start at g==0, stop at g==3); chains interleave across
                # banks, which PSUM has_written tracking handles.
                pss = [pp.tile([128, JT], F32, name=f"ps{j}", tag="ps",
                               bufs=8) for j in range(N_J)]
                for g in range(N_IG):
                    xt = xp.tile([C, IG, T], F32R, name=f"xt{g}", tag="xt")
                    if b == 0:
                        # piecewise first-batch tiles: the first matmul
                        # chain starts after 1 MiB instead of 4 MiB
                        for m in range(IG):
                            nc.sync.dma_start(
                                out=xt[:, m, :],
                                in_=x[b, g * IG + m].rearrange(
                                    "c t -> c t").bitcast(F32R))
                    else:
                        nc.sync.dma_start(
                            out=xt[:, :, :],
                            in_=x[b, g * IG:(g + 1) * IG].rearrange(
                                "i c t -> c i t").bitcast(F32R))
                    for m in range(IG):
                        i = g * IG + m
                        for j in range(N_J):
                            nc.tensor.matmul(
                                pss[j][:, :], wbd[:, i, :],
                                xt[:, m, j * JT:(j + 1) * JT],
                                start=(i == 0), stop=(i == I - 1),
                                skip_group_check=True)
                for j in range(N_J):
                    sl = slice(j * JT, (j + 1) * JT)
                    nc.vector.tensor_scalar_mul(out_sb[:, sl], pss[j][:, :],
                                                scale[:, 0:1])
            else:
                xts = []
                for g in range(N_IG):
                    xt = xp.tile([C, IG, T], mm_dt, name=f"xt{g}", tag="xt")
                    if b == 0 and g == 0:
                        # piecewise first tile: the first matmul chain can
                        # start after 1 MiB instead of 4 MiB (single-shot
                        # pipeline fill)
                        for m in range(IG):
                            nc.sync.dma_start(
                                out=xt[:, m, :],
                                in_=x[b, g * IG + m].rearrange("c t -> c t"))
                    else:
                        nc.sync.dma_start(
                            out=xt[:, :, :],
                            in_=x[b, g * IG:(g + 1) * IG].rearrange(
                                "i c t -> c i t"))
                    xts.append(xt)
                # j-outer: each (b,j) fills one dense (128,512) psum bank;
                # the 4 i-groups land in distinct 32-row col-strips of the
                # PE array (tile_position), so consecutive groups overlap
                # in the array. One full-width copy per (b,j).
                for j in range(N_J):
                    sl = slice(j * JT, (j + 1) * JT)
                    ps = pp.tile([128, JT], F32, name="psc", tag="ps",
                                 bufs=8)
                    # half-chain strip interleave: strips switch every 2
                    # matmuls so consecutive instructions overlap in
                    # different col-strips of the PE array
                    order = [(g, m) for half in range(2)
                             for g in range(N_IG)
                             for m in (half * 2, half * 2 + 1)]
                    for g, m in order:
                        i = g * IG + m
                        nc.tensor.matmul(
                            ps[g * 32:(g + 1) * 32, :],
                            wbd4[:, i, :], xts[g][:, m, sl],
                            start=(m == 0), stop=(m == IG - 1),
                            tile_position=(0, g * 32))
                    nc.vector.tensor_scalar_mul(out_sb[:, sl], ps[:, :],
                                                scale[:, 0:1])
            # out-DMA on the ACT HWDGE ring: its sem wait (drain copies)
            # must not stall the SP sequencer, which streams the next
            # batch's input loads
            nc.scalar.dma_start(out=o[b, :, :], in_=out_sb[:, :])


def _build(mm_dt=F32R):
    nc = bacc.Bacc()
    x = nc.declare_dram_parameter("x", [BPC, I, C, T], F32, isOutput=False)
    w = nc.declare_dram_parameter("w", [OUT_CH, 1, C, 1], F32, isOutput=False)
    o = nc.declare_dram_parameter("o", [BPC, OUT_CH, T], F32, isOutput=True)

    with tile.TileContext(nc) as tc:
        _body(nc, tc, x, w, o, mm_dt=mm_dt)

    if not nc.is_finalized():
        nc.finalize()
    return nc


def _get_nc():
    if "nc" not in _CACHE:
        _CACHE["nc"] = _build()
    return _CACHE["nc"]


def _run(x, weight, **kw):
    assert x.shape == (B, I, C, T) and x.dtype == np.float32
    assert weight.shape == (OUT_CH, 1, C, 1) and weight.dtype == np.float32
    nc = _get_nc()
    in_maps = [
        {"x": np.ascontiguousarray(x[k * BPC:(k + 1) * BPC]), "w": weight}
        for k in range(N_CORES)
    ]
    res = run_bass_kernel_spmd(nc, in_maps, list(range(N_CORES)), **kw)
    out = np.concatenate([res.results[k]["o"] for k in range(N_CORES)], axis=0)
    return out.reshape(B, OUT_CH, 1, T), res


def kernel(x, weight):
    out, _ = _run(x, weight)
    return out


# revision 11
# speedup vs baseline: 1.6078x; 1.6078x over previous
"""Trainium2 Bass kernel for depthwise-spatial-conv:
out[b, i*D+d, 0, t] = sum_c maxnorm(w)[i*D+d, c] * x[b, i, c, t]

Sharding: data-parallel over batch (B=32 -> 4 per core across 8 cores),
weight replicated on every core.

Per core, each (b, i) is a tiny (8 x 128) @ (128 x 2048) matmul.
Structure: i-blocks are processed in groups of 4 via block-diagonal
(C x 32) weights, so each 4-matmul PSUM accumulation group yields a dense
(32, 512) tile at a 32-aligned partition base. The 4 groups sit in
distinct 32-col strips of the PE array (tile_position) and run
concurrently.

The matmul runs in bf16 (tolerance is 2e-2; bf16 lands ~1e-3): fp32
would stream the moving operand at 4 cycles/row and make PE the
bottleneck; bf16 streams at 1 cycle/row, leaving the kernel bound by
the HBM read of x (~358 GB/s/core). float32r was measured 1.85x WORSE
than fp32 (its fp32_mode=HIGH path gets no HAM warm-up credit and
streams at 4 cyc/row, and it forbids 32-col-strip concurrency).

Cast modes:
  bf16_swdge - fp32->bf16 conversion inline in the x DMA (SWDGE/gpsimd
               path does dtype casts at stream rate).
  bf16_dve   - fp32 staged load (HWDGE) + DVE tensor_copy cast.
  fp32       - original exact-fp32 path (reference/baseline).
"""
import numpy as np

import concourse.bacc as bacc
import concourse.mybir as mybir
import concourse.tile as tile
from concourse.bass_utils import run_bass_kernel_spmd
from concourse.masks import make_identity

F32 = mybir.dt.float32
BF16 = mybir.dt.bfloat16

B, I, C, T, D = 32, 16, 128, 2048, 8
OUT_CH = I * D  # 128
N_CORES = 8
BPC = B // N_CORES  # batches per core
IG = 4            # i-blocks per DMA tile and per psum group
N_IG = I // IG    # 4
JT = 512          # matmul moving free-dim chunk (psum bank limit f32 out)
N_J = T // JT     # 4

MODE = "bf16_swdge"

_CACHE = {}


def _preprocess_weights(nc, wp, pp, w):
    """DMA w, transpose to wT[c, oc] (unscaled), and compute the torch
    renorm(p=2, dim=0, maxnorm=1) scale as a per-out-channel (128,1)
    vector. The scale is applied during the PSUM-drain copies, so the
    sqrt/ACT-table chain stays off the first-matmul critical path."""
    w_sb = wp.tile([OUT_CH, C], F32)
    # ACT ring: keep the SP ring free so the first x load issues immediately
    nc.scalar.dma_start(out=w_sb[:, :], in_=w[:, 0, :, 0])
    sq = wp.tile([OUT_CH, C], F32)
    nc.vector.tensor_mul(sq[:, :], w_sb[:, :], w_sb[:, :])
    norm2 = wp.tile([OUT_CH, 1], F32)
    nc.vector.reduce_sum(out=norm2[:, :], in_=sq[:, :],
                         axis=mybir.AxisListType.X)
    norm = wp.tile([OUT_CH, 1], F32)
    nc.scalar.activation(out=norm[:, :], in_=norm2[:, :],
                         func=mybir.ActivationFunctionType.Sqrt,
                         bias=0.0, scale=1.0)
    nc.vector.tensor_scalar_max(norm[:, :], norm[:, :], 1e-12)
    inv = wp.tile([OUT_CH, 1], F32)
    nc.vector.reciprocal(inv[:, :], norm[:, :])
    nc.vector.tensor_scalar_min(inv[:, :], inv[:, :], 1.0)
    ident = wp.tile([128, 128], F32)
    make_identity(nc, ident[:, :])
    pt = pp.tile([128, 128], F32, tag="ps", bufs=8)
    nc.tensor.transpose(pt[:, :], w_sb[:, :], ident[:, :])
    return pt, inv


def _blockdiag4(nc, wp, wT, dtype, name):
    """t[:, i, :] is (C, 32): cols [8*(i%4), 8*(i%4)+8) = wT[:, 8i:8i+8),
    zero elsewhere (DVE casts fp32->dtype during the copies). A 4-matmul
    accumulation over i in one group fills a dense (32, JT) psum tile."""
    t = wp.tile([C, I, 32], dtype, name=name)
    nc.vector.memset(t[:, :, :], 0.0)
    for i in range(I):
        m = i % IG
        nc.vector.tensor_copy(t[:, i, m * D:(m + 1) * D],
                              wT[:, i * D:(i + 1) * D])
    return t


def _warmup_pe(nc, wp, pp):
    """HAM throttles a cold PE to 1.2 GHz until ~3.4us of sustained
    matmul activity. Burn that window during the initial DMA fill with
    dummy matmuls so the real stream starts at full clock."""
    wdum = wp.tile([128, 128], F32, name="wdum")
    nc.vector.memset(wdum[:, :], 0.5)
    psd = pp.tile([32, 128], F32, name="psd", tag="ps", bufs=8)
    for _ in range(12):
        nc.tensor.matmul(psd[:, :], wdum[:, :32], wdum[:, :],
                         start=True, stop=True)


def _mm_j_loop(nc, op, pp, xts, wbd4, scale, out_sb):
    """j-outer: each (b,j) fills one dense (128,512) psum bank; the 4
    i-groups land in distinct 32-col strips of the PE array
    (tile_position), so groups overlap in the array. One full-width
    scale-copy per (b,j) drains PSUM -> out_sb."""
    for j in range(N_J):
        sl = slice(j * JT, (j + 1) * JT)
        ps = pp.tile([128, JT], F32, name="psc", tag="ps", bufs=8)
        # half-chain strip interleave: strips switch every 2 matmuls so
        # consecutive instructions overlap in different col-strips
        order = [(g, m) for half in range(2)
                 for g in range(N_IG)
                 for m in (half * 2, half * 2 + 1)]
        for g, m in order:
            i = g * IG + m
            nc.tensor.matmul(
                ps[g * 32:(g + 1) * 32, :],
                wbd4[:, i, :], xts[g][:, m, sl],
                start=(m == 0), stop=(m == IG - 1),
                tile_position=(0, g * 32))
        nc.vector.tensor_scalar_mul(out_sb[:, sl], ps[:, :],
                                    scale[:, 0:1])


def _body(nc, tc, x, w, o, mode=None):
    mode = mode or MODE
    mm_dt = F32 if mode == "fp32" else BF16
    xt_bufs = {"fp32": 5, "bf16_swdge": 8, "bf16_dve": 5}[mode]
    with tc.tile_pool(name="wp", bufs=1) as wp, \
         tc.tile_pool(name="xp", bufs=xt_bufs) as xp, \
         tc.tile_pool(name="sp", bufs=2) as sp, \
         tc.tile_pool(name="op", bufs=3) as op, \
         tc.tile_pool(name="pp", bufs=1, space="PSUM") as pp:
        wT, scale = _preprocess_weights(nc, wp, pp, w)
        wbd4 = _blockdiag4(nc, wp, wT, mm_dt, "wbd4")
        _warmup_pe(nc, wp, pp)

        for b in range(BPC):
            out_sb = op.tile([OUT_CH, T], F32, name="out_sb", tag="ob")
            xts = []
            for g in range(N_IG):
                xt = xp.tile([C, IG, T], mm_dt, name=f"xt{g}", tag="xt")
                src = x[b, g * IG:(g + 1) * IG].rearrange("i c t -> c i t")
                if mode == "bf16_swdge":
                    # cast-in-DMA: SWDGE converts fp32->bf16 at stream
                    # rate; HBM read traffic is unchanged (fp32 source)
                    if b == 0:
                        for m in range(IG):
                            nc.gpsimd.dma_start(
                                out=xt[:, m, :],
                                in_=x[b, g * IG + m].rearrange("c t -> c t"))
                    else:
                        nc.gpsimd.dma_start(out=xt[:, :, :], in_=src)
                elif mode == "bf16_dve":
                    xs = sp.tile([C, IG, T], F32, name=f"xs{g}", tag="xs")
                    if b == 0:
                        for m in range(IG):
                            nc.sync.dma_start(
                                out=xs[:, m, :],
                                in_=x[b, g * IG + m].rearrange("c t -> c t"))
                    else:
                        nc.sync.dma_start(out=xs[:, :, :], in_=src)
                    for m in range(IG):
                        nc.vector.tensor_copy(xt[:, m, :], xs[:, m, :])
                else:
                    if b == 0 and g == 0:
                        # piecewise first tile: the first matmul chain
                        # starts after 1 MiB instead of 4 MiB
                        for m in range(IG):
                            nc.sync.dma_start(
                                out=xt[:, m, :],
                                in_=x[b, g * IG + m].rearrange("c t -> c t"))
                    else:
                        nc.sync.dma_start(out=xt[:, :, :], in_=src)
                xts.append(xt)
            _mm_j_loop(nc, op, pp, xts, wbd4, scale, out_sb)
            # out-DMA on the ACT HWDGE ring: its sem wait (drain copies)
            # must not stall the SP sequencer streaming the input loads
            nc.scalar.dma_start(out=o[b, :, :], in_=out_sb[:, :])


def _build(mode=None):
    nc = bacc.Bacc()
    x = nc.declare_dram_parameter("x", [BPC, I, C, T], F32, isOutput=False)
    w = nc.declare_dram_parameter("w", [OUT_CH, 1, C, 1], F32, isOutput=False)
    o = nc.declare_dram_parameter("o", [BPC, OUT_CH, T], F32, isOutput=True)

    with tile.TileContext(nc) as tc:
        _body(nc, tc, x, w, o, mode=mode)

    if not nc.is_finalized():
        nc.finalize()
    return nc


def _get_nc():
    if "nc" not in _CACHE:
        _CACHE["nc"] = _build()
    return _CACHE["nc"]


def _run(x, weight, **kw):
    assert x.shape == (B, I, C, T) and x.dtype == np.float32
    assert weight.shape == (OUT_CH, 1, C, 1) and weight.dtype == np.float32
    nc = _get_nc()
    in_maps = [
        {"x": np.ascontiguousarray(x[k * BPC:(k + 1) * BPC]), "w": weight}
        for k in range(N_CORES)
    ]
    res = run_bass_kernel_spmd(nc, in_maps, list(range(N_CORES)), **kw)
    out = np.concatenate([res.results[k]["o"] for k in range(N_CORES)], axis=0)
    return out.reshape(B, OUT_CH, 1, T), res


def kernel(x, weight):
    out, _ = _run(x, weight)
    return out


# revision 13
# speedup vs baseline: 1.8320x; 1.1394x over previous
"""Trainium2 Bass kernel for depthwise-spatial-conv:
out[b, i*D+d, 0, t] = sum_c maxnorm(w)[i*D+d, c] * x[b, i, c, t]

Sharding: data-parallel over batch (B=32 -> 4 per core across 8 cores),
weight replicated on every core.

Per core, each (b, i) is a tiny (8 x 128) @ (128 x 2048) matmul.
Structure: i-blocks are processed in groups of 4 via block-diagonal
(C x 32) weights, so each 4-matmul PSUM accumulation group yields a dense
(32, 512) tile at a 32-aligned partition base. The 4 groups sit in
distinct 32-col strips of the PE array (tile_position) and run
concurrently.

The matmul runs in bf16 (tolerance is 2e-2; bf16 lands ~1e-3): fp32
would stream the moving operand at 4 cycles/row and make PE the
bottleneck; bf16 streams at 1 cycle/row, leaving the kernel bound by
the HBM read of x (~358 GB/s/core). float32r was measured 1.85x WORSE
than fp32 (its fp32_mode=HIGH path gets no HAM warm-up credit and
streams at 4 cyc/row, and it forbids 32-col-strip concurrency).

Cast modes:
  bf16_swdge - fp32->bf16 conversion inline in the x DMA (SWDGE/gpsimd
               path does dtype casts at stream rate).
  bf16_dve   - fp32 staged load (HWDGE) + DVE tensor_copy cast.
  fp32       - original exact-fp32 path (reference/baseline).
"""
import numpy as np

import concourse.bacc as bacc
import concourse.mybir as mybir
import concourse.tile as tile
from concourse.bass_utils import run_bass_kernel_spmd
from concourse.masks import make_identity

F32 = mybir.dt.float32
BF16 = mybir.dt.bfloat16

B, I, C, T, D = 32, 16, 128, 2048, 8
OUT_CH = I * D  # 128
N_CORES = 8
BPC = B // N_CORES  # batches per core
IG = 4            # i-blocks per DMA tile and per psum group
N_IG = I // IG    # 4
JT = 512          # matmul moving free-dim chunk (psum bank limit f32 out)
N_J = T // JT     # 4

MODE = "bf16_lin"

_CACHE = {}


def _preprocess_weights(nc, wp, pp, w):
    """DMA w, transpose to wT[c, oc] (unscaled), and compute the torch
    renorm(p=2, dim=0, maxnorm=1) scale as a per-out-channel (128,1)
    vector. The scale is applied during the PSUM-drain copies, so the
    sqrt/ACT-table chain stays off the first-matmul critical path."""
    w_sb = wp.tile([OUT_CH, C], F32)
    # ACT ring: keep the SP ring free so the first x load issues immediately
    nc.scalar.dma_start(out=w_sb[:, :], in_=w[:, 0, :, 0])
    sq = wp.tile([OUT_CH, C], F32)
    nc.vector.tensor_mul(sq[:, :], w_sb[:, :], w_sb[:, :])
    norm2 = wp.tile([OUT_CH, 1], F32)
    nc.vector.reduce_sum(out=norm2[:, :], in_=sq[:, :],
                         axis=mybir.AxisListType.X)
    norm = wp.tile([OUT_CH, 1], F32)
    nc.scalar.activation(out=norm[:, :], in_=norm2[:, :],
                         func=mybir.ActivationFunctionType.Sqrt,
                         bias=0.0, scale=1.0)
    nc.vector.tensor_scalar_max(norm[:, :], norm[:, :], 1e-12)
    inv = wp.tile([OUT_CH, 1], F32)
    nc.vector.reciprocal(inv[:, :], norm[:, :])
    nc.vector.tensor_scalar_min(inv[:, :], inv[:, :], 1.0)
    ident = wp.tile([128, 128], F32)
    make_identity(nc, ident[:, :])
    pt = pp.tile([128, 128], F32, tag="ps", bufs=8)
    nc.tensor.transpose(pt[:, :], w_sb[:, :], ident[:, :])
    return pt, inv


def _blockdiag4(nc, wp, wT, dtype, name):
    """t[:, i, :] is (C, 32): cols [8*(i%4), 8*(i%4)+8) = wT[:, 8i:8i+8),
    zero elsewhere (DVE casts fp32->dtype during the copies). A 4-matmul
    accumulation over i in one group fills a dense (32, JT) psum tile."""
    t = wp.tile([C, I, 32], dtype, name=name)
    nc.vector.memset(t[:, :, :], 0.0)
    for i in range(I):
        m = i % IG
        nc.vector.tensor_copy(t[:, i, m * D:(m + 1) * D],
                              wT[:, i * D:(i + 1) * D])
    return t


def _warmup_pe(nc, wp, pp):
    """HAM throttles a cold PE to 1.2 GHz until ~3.4us of sustained
    matmul activity. Burn that window during the initial DMA fill with
    dummy matmuls so the real stream starts at full clock."""
    wdum = wp.tile([128, 128], F32, name="wdum")
    nc.vector.memset(wdum[:, :], 0.5)
    psd = pp.tile([32, 128], F32, name="psd", tag="ps", bufs=8)
    for _ in range(12):
        nc.tensor.matmul(psd[:, :], wdum[:, :32], wdum[:, :],
                         start=True, stop=True)


def _mm_j_loop(nc, op, pp, xts, wbd4, scale, out_sb):
    """j-outer: each (b,j) fills one dense (128,512) psum bank; the 4
    i-groups land in distinct 32-col strips of the PE array
    (tile_position), so groups overlap in the array. One full-width
    scale-copy per (b,j) drains PSUM -> out_sb."""
    for j in range(N_J):
        sl = slice(j * JT, (j + 1) * JT)
        ps = pp.tile([128, JT], F32, name="psc", tag="ps", bufs=8)
        # half-chain strip interleave: strips switch every 2 matmuls so
        # consecutive instructions overlap in different col-strips
        order = [(g, m) for half in range(2)
                 for g in range(N_IG)
                 for m in (half * 2, half * 2 + 1)]
        for g, m in order:
            i = g * IG + m
            nc.tensor.matmul(
                ps[g * 32:(g + 1) * 32, :],
                wbd4[:, i, :], xts[g][:, m, sl],
                start=(m == 0), stop=(m == IG - 1),
                tile_position=(0, g * 32))
        nc.vector.tensor_scalar_mul(out_sb[:, sl], ps[:, :],
                                    scale[:, 0:1])


def _body_bf16_lin(nc, tc, x, w, o):
    """Linear-load + DVE-cast pipeline, bound by the x HBM read:

    - x is loaded per (b, i) as fully-linear 1 MiB (C, T) fp32 tiles on
      the SP HWDGE ring (measured ~356 GB/s/core with bufs>=8 -- deep
      queue of linear descriptors).
    - DVE casts each tile to bf16 right as it lands (~1.5 us/tile,
      hides under the ~3 us/tile DMA).
    - PE: g-outer sections -- each group of 4 bf16 tiles is consumed by
      its 16 matmuls (4 m x 4 psum banks) immediately, so tiles free
      after their own section and never gate the DMA stream on
      batch-tail compute. The 4 psum banks accumulate across sections
      in 32-row strips (start at g==0's m==0, stop at g==3's m==3 per
      strip); bank chains interleave, which per-element has_written
      tracking handles.
    """
    with tc.tile_pool(name="wp", bufs=1) as wp, \
         tc.tile_pool(name="sp", bufs=8) as sp, \
         tc.tile_pool(name="xp", bufs=8) as xp, \
         tc.tile_pool(name="op", bufs=3) as op, \
         tc.tile_pool(name="pp", bufs=1, space="PSUM") as pp:
        wT, scale = _preprocess_weights(nc, wp, pp, w)
        wbd4 = _blockdiag4(nc, wp, wT, BF16, "wbd4")
        _warmup_pe(nc, wp, pp)

        for b in range(BPC):
            out_sb = op.tile([OUT_CH, T], F32, name="out_sb", tag="ob")
            pss = [pp.tile([128, JT], F32, name=f"ps{j}", tag="ps", bufs=8)
                   for j in range(N_J)]
            for g in range(N_IG):
                xbs = []
                for m in range(IG):
                    i = g * IG + m
                    xs = sp.tile([C, T], F32, name="xs", tag="xs")
                    nc.sync.dma_start(out=xs[:, :], in_=x[b, i])
                    xb = xp.tile([C, T], BF16, name="xb", tag="xb")
                    nc.vector.tensor_copy(xb[:, :], xs[:, :])
                    xbs.append(xb)
                for m in range(IG):
                    i = g * IG + m
                    for j in range(N_J):
                        nc.tensor.matmul(
                            pss[j][g * 32:(g + 1) * 32, :],
                            wbd4[:, i, :],
                            xbs[m][:, j * JT:(j + 1) * JT],
                            start=(m == 0), stop=(m == IG - 1),
                            tile_position=(0, g * 32),
                            skip_group_check=True)
            for j in range(N_J):
                sl = slice(j * JT, (j + 1) * JT)
                nc.vector.tensor_scalar_mul(out_sb[:, sl], pss[j][:, :],
                                            scale[:, 0:1])
            # out-DMA on the ACT HWDGE ring: its sem wait (drain copies)
            # must not stall the SP sequencer streaming the input loads
            nc.scalar.dma_start(out=o[b, :, :], in_=out_sb[:, :])


def _body(nc, tc, x, w, o, mode=None):
    mode = mode or MODE
    if mode == "bf16_lin":
        return _body_bf16_lin(nc, tc, x, w, o)
    mm_dt = F32 if mode == "fp32" else BF16
    xt_bufs = {"fp32": 5, "bf16_swdge": 8, "bf16_dve": 5}[mode]
    with tc.tile_pool(name="wp", bufs=1) as wp, \
         tc.tile_pool(name="xp", bufs=xt_bufs) as xp, \
         tc.tile_pool(name="sp", bufs=2) as sp, \
         tc.tile_pool(name="op", bufs=3) as op, \
         tc.tile_pool(name="pp", bufs=1, space="PSUM") as pp:
        wT, scale = _preprocess_weights(nc, wp, pp, w)
        wbd4 = _blockdiag4(nc, wp, wT, mm_dt, "wbd4")
        _warmup_pe(nc, wp, pp)

        for b in range(BPC):
            out_sb = op.tile([OUT_CH, T], F32, name="out_sb", tag="ob")
            xts = []
            for g in range(N_IG):
                xt = xp.tile([C, IG, T], mm_dt, name=f"xt{g}", tag="xt")
                src = x[b, g * IG:(g + 1) * IG].rearrange("i c t -> c i t")
                if mode == "bf16_swdge":
                    # cast-in-DMA: SWDGE converts fp32->bf16 at stream
                    # rate; HBM read traffic is unchanged (fp32 source)
                    if b == 0:
                        for m in range(IG):
                            nc.gpsimd.dma_start(
                                out=xt[:, m, :],
                                in_=x[b, g * IG + m].rearrange("c t -> c t"))
                    else:
                        nc.gpsimd.dma_start(out=xt[:, :, :], in_=src)
                elif mode == "bf16_dve":
                    xs = sp.tile([C, IG, T], F32, name=f"xs{g}", tag="xs")
                    if b == 0:
                        for m in range(IG):
                            nc.sync.dma_start(
                                out=xs[:, m, :],
                                in_=x[b, g * IG + m].rearrange("c t -> c t"))
                    else:
                        nc.sync.dma_start(out=xs[:, :, :], in_=src)
                    for m in range(IG):
                        nc.vector.tensor_copy(xt[:, m, :], xs[:, m, :])
                else:
                    if b == 0 and g == 0:
                        # piecewise first tile: the first matmul chain
                        # starts after 1 MiB instead of 4 MiB
                        for m in range(IG):
                            nc.sync.dma_start(
                                out=xt[:, m, :],
                                in_=x[b, g * IG + m].rearrange("c t -> c t"))
                    else:
                        nc.sync.dma_start(out=xt[:, :, :], in_=src)
                xts.append(xt)
            _mm_j_loop(nc, op, pp, xts, wbd4, scale, out_sb)
            # out-DMA on the ACT HWDGE ring: its sem wait (drain copies)
            # must not stall the SP sequencer streaming the input loads
            nc.scalar.dma_start(out=o[b, :, :], in_=out_sb[:, :])


def _build(mode=None):
    nc = bacc.Bacc()
    x = nc.declare_dram_parameter("x", [BPC, I, C, T], F32, isOutput=False)
    w = nc.declare_dram_parameter("w", [OUT_CH, 1, C, 1], F32, isOutput=False)
    o = nc.declare_dram_parameter("o", [BPC, OUT_CH, T], F32, isOutput=True)

    with tile.TileContext(nc) as tc:
        _body(nc, tc, x, w, o, mode=mode)

    if not nc.is_finalized():
        nc.finalize()
    return nc


def _get_nc():
    if "nc" not in _CACHE:
        _CACHE["nc"] = _build()
    return _CACHE["nc"]


def _run(x, weight, **kw):
    assert x.shape == (B, I, C, T) and x.dtype == np.float32
    assert weight.shape == (OUT_CH, 1, C, 1) and weight.dtype == np.float32
    nc = _get_nc()
    in_maps = [
        {"x": np.ascontiguousarray(x[k * BPC:(k + 1) * BPC]), "w": weight}
        for k in range(N_CORES)
    ]
    res = run_bass_kernel_spmd(nc, in_maps, list(range(N_CORES)), **kw)
    out = np.concatenate([res.results[k]["o"] for k in range(N_CORES)], axis=0)
    return out.reshape(B, OUT_CH, 1, T), res


def kernel(x, weight):
    out, _ = _run(x, weight)
    return out
